# revision 15
# baseline (speedup 1.0000x reference)
"""BlockAttnRes Trainium2 kernel.

Computes, for V = stack([*blocks, partial_block]) (shape [8, B, T, D]) and two
(pseudo_query, norm_weight) pairs:
    K = rmsnorm(V, w);  logits = (q . K) / sqrt(D);  attn = softmax_n(logits)
    out = sum_n attn[n] * V[n]
Key algebraic identity used: q . K = rsqrt(mean(V^2)+eps) * ((q*w) . V), so K is
never materialized.  Sharding: pure data parallel over the flattened (B,T) axis
(8192 rows -> 1024 rows per core), [D] params replicated.

Per-core engine split (layout: partition = t, free = d):
  ACT  : sum-of-squares per (n,t) via fused Square+accum_out
  DVE  : the two (qw . V) dot products via fused tensor_tensor_reduce
  DVE/ACT: softmax over the n=8 axis on tiny [128, 8] tiles
  PE   : out = sum_n diag(attn_n) @ V_n  (f32r matmuls, PSUM accumulation)
  ACT  : PSUM -> SBUF copy, then DMA out
"""

import math
import numpy as np

N_CORES = 8
NB = 8            # blocks incl. partial
B, T, D = 2, 4096, 2048
BT = B * T        # 8192 flattened rows
TSH = BT // N_CORES  # 1024 rows per core
P = 128           # partitions / t-tile size
NT = TSH // P     # 8 t-tiles per core
EPS = 1e-6
MM_N = 512        # max fp32 moving free dim per matmul

_CACHE = {}


def _build(reps=1):
    from contextlib import ExitStack

    import concourse.bass as bass
    import concourse.tile as tile
    from concourse import bacc, mybir

    f32 = mybir.dt.float32
    f32r = mybir.dt.float32r
    bf16 = mybir.dt.bfloat16
    f16 = mybir.dt.float16
    Alu = mybir.AluOpType
    Act = mybir.ActivationFunctionType

    nc = bacc.Bacc("TRN2", target_bir_lowering=False, debug=False,
                   num_devices=N_CORES)

    v = nc.dram_tensor("v", [NB, TSH, D], f32, kind="ExternalInput").ap()
    qwa_d = nc.dram_tensor("qwa", [P, D], f32, kind="ExternalInput").ap()
    qwm_d = nc.dram_tensor("qwm", [P, D], f32, kind="ExternalInput").ap()
    ident_d = nc.dram_tensor("ident", [P, P], f16, kind="ExternalInput").ap()
    oa = nc.dram_tensor("oa", [TSH, D], f32, kind="ExternalOutput").ap()
    om = nc.dram_tensor("om", [TSH, D], f32, kind="ExternalOutput").ap()

    with tile.TileContext(nc) as tc, ExitStack() as ctx:
        const_p = ctx.enter_context(tc.tile_pool(name="const", bufs=1))
        x_p = ctx.enter_context(tc.tile_pool(name="x", bufs=10))
        xb_p = ctx.enter_context(tc.tile_pool(name="xb", bufs=12))
        scr_p = ctx.enter_context(tc.tile_pool(name="scr", bufs=2))
        stat_p = ctx.enter_context(tc.tile_pool(name="stat", bufs=4))
        diag_p = ctx.enter_context(tc.tile_pool(name="diag", bufs=20))
        out_p = ctx.enter_context(tc.tile_pool(name="out", bufs=2))
        psum_p = ctx.enter_context(
            tc.tile_pool(name="psum", bufs=2, space=bass.MemorySpace.PSUM))

        qwa = const_p.tile([P, D], f32)
        qwm = const_p.tile([P, D], f32)
        ident = const_p.tile([P, P], f16)
        nc.sync.dma_start(out=qwa[:], in_=qwa_d[:])
        nc.sync.dma_start(out=qwm[:], in_=qwm_d[:])
        nc.sync.dma_start(out=ident[:], in_=ident_d[:])

        rep_cm = tc.For_i(0, reps, 1) if reps > 1 else None
        if rep_cm is not None:
            rep_cm.__enter__()
        for i in range(NT):
            t0 = i * P
            # ---- load the 8 V tiles, make bf16 copies (ACT) --------------
            xg = []
            xbg = []
            for n in range(NB):
                xt = x_p.tile([P, D], f32, tag="x")
                nc.sync.dma_start(out=xt[:], in_=v[n, t0:t0 + P, :])
                xg.append(xt)
                xb = xb_p.tile([P, D], f16, tag="xb")
                nc.scalar.copy(xb[:], xt[:])
                xbg.append(xb)

            # ---- reductions over d: ss (ACT), dots (DVE) -----------------
            ss = stat_p.tile([P, NB], f32, tag="ss")
            da = stat_p.tile([P, NB], f32, tag="da")
            dm = stat_p.tile([P, NB], f32, tag="dm")
            for n in range(NB):
                sa = scr_p.tile([P, D], bf16, tag="sa")
                nc.scalar.activation(sa[:], xg[n][:], Act.Square,
                                     accum_out=ss[:, n:n + 1])
                sv = scr_p.tile([P, D], bf16, tag="sv")
                nc.vector.scalar_tensor_tensor(
                    out=sv[:], in0=xg[n][:], scalar=1.0, in1=qwa[:],
                    op0=Alu.mult, op1=Alu.mult, accum_out=da[:, n:n + 1])
                sv2 = scr_p.tile([P, D], bf16, tag="sv2")
                nc.vector.scalar_tensor_tensor(
                    out=sv2[:], in0=xg[n][:], scalar=1.0, in1=qwm[:],
                    op0=Alu.mult, op1=Alu.mult, accum_out=dm[:, n:n + 1])

            # ---- rms factor: 1/sqrt(ss/D + eps) --------------------------
            u = stat_p.tile([P, NB], f32, tag="u")
            nc.vector.tensor_scalar(u[:], ss[:], 1.0 / D, EPS,
                                    Alu.mult, Alu.add)
            sq = stat_p.tile([P, NB], f32, tag="sq")
            nc.scalar.sqrt(sq[:], u[:])
            rms = stat_p.tile([P, NB], f32, tag="rms")
            nc.vector.reciprocal(rms[:], sq[:])

            # ---- softmax over n, then diag(attn) tiles -------------------
            diags = {}
            for kind, dvec in (("a", da), ("m", dm)):
                lg = stat_p.tile([P, NB], f32, tag=f"lg{kind}")
                nc.vector.tensor_tensor(out=lg[:], in0=dvec[:], in1=rms[:],
                                        op=Alu.mult)
                negm = stat_p.tile([P, 1], f32, tag=f"negm{kind}")
                nc.vector.tensor_reduce(out=negm[:], in_=lg[:],
                                        axis=mybir.AxisListType.X,
                                        op=Alu.max, negate=True)
                e = stat_p.tile([P, NB], f32, tag=f"e{kind}")
                se = stat_p.tile([P, 1], f32, tag=f"se{kind}")
                nc.scalar.activation(e[:], lg[:], Act.Exp,
                                     bias=negm[:, 0:1], scale=1.0,
                                     accum_out=se[:])
                r = stat_p.tile([P, 1], f32, tag=f"r{kind}")
                nc.vector.reciprocal(r[:], se[:])
                att = stat_p.tile([P, NB], f32, tag=f"att{kind}")
                nc.vector.tensor_scalar(att[:], e[:], r[:, 0:1], None,
                                        Alu.mult)
                dl = []
                for n in range(NB):
                    dg = diag_p.tile([P, P], f16, tag="dg")
                    nc.vector.tensor_scalar(dg[:], ident[:],
                                            att[:, n:n + 1], None, Alu.mult)
                    dl.append(dg)
                diags[kind] = dl

            # ---- PE: out = sum_n diag(attn_n) @ x_n ----------------------
            for kind, odram in (("a", oa), ("m", om)):
                ps = psum_p.tile([P, D], f32, tag="ps")
                for n in range(NB):
                    for j in range(D // MM_N):
                        nc.tensor.matmul(
                            ps[:, j * MM_N:(j + 1) * MM_N],
                            diags[kind][n][:],
                            xbg[n][:, j * MM_N:(j + 1) * MM_N],
                            start=(n == 0), stop=(n == NB - 1))
                ob = out_p.tile([P, D], f32, tag="ob")
                nc.scalar.copy(ob[:], ps[:])
                nc.sync.dma_start(out=odram[t0:t0 + P, :], in_=ob[:])
        if rep_cm is not None:
            rep_cm.__exit__(None, None, None)

    nc.compile()
    return nc


def _get_nc():
    if "nc" not in _CACHE:
        _CACHE["nc"] = _build()
    return _CACHE["nc"]


def kernel(blocks, partial_block, pseudo_query_attn, pseudo_query_mlp,
           norm_weight_attn, norm_weight_mlp):
    from concourse.bass_utils import run_bass_kernel_spmd

    nc = _get_nc()

    f32 = np.float32
    V = np.concatenate(
        [np.asarray(blocks, f32).reshape(NB - 1, BT, D),
         np.asarray(partial_block, f32).reshape(1, BT, D)], axis=0)

    scale = 1.0 / math.sqrt(D)
    qwa = (np.asarray(pseudo_query_attn, f32)
           * np.asarray(norm_weight_attn, f32) * scale)
    qwm = (np.asarray(pseudo_query_mlp, f32)
           * np.asarray(norm_weight_mlp, f32) * scale)
    qwa_rep = np.ascontiguousarray(np.broadcast_to(qwa, (P, D)))
    qwm_rep = np.ascontiguousarray(np.broadcast_to(qwm, (P, D)))
    import ml_dtypes
    ident = np.eye(P, dtype=np.float16)

    in_maps = []
    for c in range(N_CORES):
        sl = np.ascontiguousarray(V[:, c * TSH:(c + 1) * TSH, :])
        in_maps.append({"v": sl, "qwa": qwa_rep, "qwm": qwm_rep,
                       "ident": ident})

    import os
    kw = {}
    if os.environ.get("KERNEL_TRACE"):
        kw = {"trace": True, "tmpdir": os.environ.get("KERNEL_TRACE_DIR")}
    res = run_bass_kernel_spmd(nc, in_maps, list(range(N_CORES)), **kw)
    _CACHE["last_result"] = res

    ha = np.concatenate([res.results[c]["oa"] for c in range(N_CORES)],
                        axis=0).reshape(B, T, D)
    hm = np.concatenate([res.results[c]["om"] for c in range(N_CORES)],
                        axis=0).reshape(B, T, D)
    return (ha, hm)


# revision 16
# speedup vs baseline: 1.0330x; 1.0330x over previous
"""BlockAttnRes Trainium2 kernel.

Computes, for V = stack([*blocks, partial_block]) (shape [8, B, T, D]) and two
(pseudo_query, norm_weight) pairs:
    K = rmsnorm(V, w);  logits = (q . K) / sqrt(D);  attn = softmax_n(logits)
    out = sum_n attn[n] * V[n]
Key algebraic identity used: q . K = rsqrt(mean(V^2)+eps) * ((q*w) . V), so K is
never materialized.  Sharding: pure data parallel over the flattened (B,T) axis
(8192 rows -> 1024 rows per core), [D] params replicated.

Per-core engine split (layout: partition = t, free = d):
  ACT  : sum-of-squares per (n,t) via fused Square+accum_out
  DVE  : the two (qw . V) dot products via fused tensor_tensor_reduce
  DVE/ACT: softmax over the n=8 axis on tiny [128, 8] tiles
  PE   : out = sum_n diag(attn_n) @ V_n  (f32r matmuls, PSUM accumulation)
  ACT  : PSUM -> SBUF copy, then DMA out
"""

import math
import numpy as np

N_CORES = 8
NB = 8            # blocks incl. partial
B, T, D = 2, 4096, 2048
BT = B * T        # 8192 flattened rows
TSH = BT // N_CORES  # 1024 rows per core
P = 128           # partitions / t-tile size
NT = TSH // P     # 8 t-tiles per core
EPS = 1e-6
MM_N = 512        # max fp32 moving free dim per matmul

_CACHE = {}


def _build(reps=1):
    from contextlib import ExitStack

    import concourse.bass as bass
    import concourse.tile as tile
    from concourse import bacc, mybir

    f32 = mybir.dt.float32
    f32r = mybir.dt.float32r
    bf16 = mybir.dt.bfloat16
    f16 = mybir.dt.float16
    Alu = mybir.AluOpType
    Act = mybir.ActivationFunctionType

    nc = bacc.Bacc("TRN2", target_bir_lowering=False, debug=False,
                   num_devices=N_CORES)

    v = nc.dram_tensor("v", [NB, TSH, D], f32, kind="ExternalInput").ap()
    qwa_d = nc.dram_tensor("qwa", [P, D], f32, kind="ExternalInput").ap()
    qwm_d = nc.dram_tensor("qwm", [P, D], f32, kind="ExternalInput").ap()
    ident_d = nc.dram_tensor("ident", [P, P], f16, kind="ExternalInput").ap()
    oa = nc.dram_tensor("oa", [TSH, D], f32, kind="ExternalOutput").ap()
    om = nc.dram_tensor("om", [TSH, D], f32, kind="ExternalOutput").ap()

    with tile.TileContext(nc) as tc, ExitStack() as ctx:
        const_p = ctx.enter_context(tc.tile_pool(name="const", bufs=1))
        x_p = ctx.enter_context(tc.tile_pool(name="x", bufs=10))
        xb_p = ctx.enter_context(tc.tile_pool(name="xb", bufs=12))
        scr_p = ctx.enter_context(tc.tile_pool(name="scr", bufs=2))
        stat_p = ctx.enter_context(tc.tile_pool(name="stat", bufs=4))
        diag_p = ctx.enter_context(tc.tile_pool(name="diag", bufs=20))
        out_p = ctx.enter_context(tc.tile_pool(name="out", bufs=2))
        psum_p = ctx.enter_context(
            tc.tile_pool(name="psum", bufs=2, space=bass.MemorySpace.PSUM))

        qwa = const_p.tile([P, D], f32)
        qwm = const_p.tile([P, D], f32)
        ident = const_p.tile([P, P], f16)
        nc.sync.dma_start(out=qwa[:], in_=qwa_d[:])
        nc.sync.dma_start(out=qwm[:], in_=qwm_d[:])
        nc.sync.dma_start(out=ident[:], in_=ident_d[:])

        rep_cm = tc.For_i(0, reps, 1) if reps > 1 else None
        if rep_cm is not None:
            rep_cm.__enter__()
        for i in range(NT):
            t0 = i * P
            # ---- load the 8 V tiles, make bf16 copies (ACT) --------------
            xg = []
            xbg = []
            for n in range(NB):
                xt = x_p.tile([P, D], f32, tag="x")
                nc.sync.dma_start(out=xt[:], in_=v[n, t0:t0 + P, :])
                xg.append(xt)
                xb = xb_p.tile([P, D], f16, tag="xb")
                nc.scalar.copy(xb[:], xt[:])
                xbg.append(xb)

            # ---- reductions over d: ss (ACT), dots (DVE) -----------------
            ss = stat_p.tile([P, NB], f32, tag="ss")
            da = stat_p.tile([P, NB], f32, tag="da")
            dm = stat_p.tile([P, NB], f32, tag="dm")
            for n in range(NB):
                sa = scr_p.tile([P, D], bf16, tag="sa")
                nc.scalar.activation(sa[:], xg[n][:], Act.Square,
                                     accum_out=ss[:, n:n + 1])
                sv = scr_p.tile([P, D], bf16, tag="sv")
                nc.vector.affine_mul_reduce(
                    out=sv[:], accum_out=da[:, n:n + 1], in0=xg[n][:],
                    in1=qwa[:], scale=1.0, bias=0.0)
                sv2 = scr_p.tile([P, D], bf16, tag="sv2")
                nc.vector.affine_mul_reduce(
                    out=sv2[:], accum_out=dm[:, n:n + 1], in0=xg[n][:],
                    in1=qwm[:], scale=1.0, bias=0.0)

            # ---- rms factor: 1/sqrt(ss/D + eps) --------------------------
            u = stat_p.tile([P, NB], f32, tag="u")
            nc.vector.tensor_scalar(u[:], ss[:], 1.0 / D, EPS,
                                    Alu.mult, Alu.add)
            sq = stat_p.tile([P, NB], f32, tag="sq")
            nc.scalar.sqrt(sq[:], u[:])
            rms = stat_p.tile([P, NB], f32, tag="rms")
            nc.vector.reciprocal(rms[:], sq[:])

            # ---- softmax over n, then diag(attn) tiles -------------------
            diags = {}
            for kind, dvec in (("a", da), ("m", dm)):
                lg = stat_p.tile([P, NB], f32, tag=f"lg{kind}")
                nc.vector.tensor_tensor(out=lg[:], in0=dvec[:], in1=rms[:],
                                        op=Alu.mult)
                negm = stat_p.tile([P, 1], f32, tag=f"negm{kind}")
                nc.vector.tensor_reduce(out=negm[:], in_=lg[:],
                                        axis=mybir.AxisListType.X,
                                        op=Alu.max, negate=True)
                e = stat_p.tile([P, NB], f32, tag=f"e{kind}")
                se = stat_p.tile([P, 1], f32, tag=f"se{kind}")
                nc.scalar.activation(e[:], lg[:], Act.Exp,
                                     bias=negm[:, 0:1], scale=1.0,
                                     accum_out=se[:])
                r = stat_p.tile([P, 1], f32, tag=f"r{kind}")
                nc.vector.reciprocal(r[:], se[:])
                att = stat_p.tile([P, NB], f32, tag=f"att{kind}")
                nc.vector.tensor_scalar(att[:], e[:], r[:, 0:1], None,
                                        Alu.mult)
                dl = []
                for n in range(NB):
                    dg = diag_p.tile([P, P], f16, tag="dg")
                    nc.vector.tensor_scalar(dg[:], ident[:],
                                            att[:, n:n + 1], None, Alu.mult)
                    dl.append(dg)
                diags[kind] = dl

            # ---- PE: out = sum_n diag(attn_n) @ x_n ----------------------
            for kind, odram in (("a", oa), ("m", om)):
                ps = psum_p.tile([P, D], f32, tag="ps")
                for n in range(NB):
                    for j in range(D // MM_N):
                        nc.tensor.matmul(
                            ps[:, j * MM_N:(j + 1) * MM_N],
                            diags[kind][n][:],
                            xbg[n][:, j * MM_N:(j + 1) * MM_N],
                            start=(n == 0), stop=(n == NB - 1))
                ob = out_p.tile([P, D], f32, tag="ob")
                nc.scalar.copy(ob[:], ps[:])
                nc.sync.dma_start(out=odram[t0:t0 + P, :], in_=ob[:])
        if rep_cm is not None:
            rep_cm.__exit__(None, None, None)

    nc.compile()
    return nc


def _get_nc():
    if "nc" not in _CACHE:
        _CACHE["nc"] = _build()
    return _CACHE["nc"]


def kernel(blocks, partial_block, pseudo_query_attn, pseudo_query_mlp,
           norm_weight_attn, norm_weight_mlp):
    from concourse.bass_utils import run_bass_kernel_spmd

    nc = _get_nc()

    f32 = np.float32
    V = np.concatenate(
        [np.asarray(blocks, f32).reshape(NB - 1, BT, D),
         np.asarray(partial_block, f32).reshape(1, BT, D)], axis=0)

    scale = 1.0 / math.sqrt(D)
    qwa = (np.asarray(pseudo_query_attn, f32)
           * np.asarray(norm_weight_attn, f32) * scale)
    qwm = (np.asarray(pseudo_query_mlp, f32)
           * np.asarray(norm_weight_mlp, f32) * scale)
    qwa_rep = np.ascontiguousarray(np.broadcast_to(qwa, (P, D)))
    qwm_rep = np.ascontiguousarray(np.broadcast_to(qwm, (P, D)))
    import ml_dtypes
    ident = np.eye(P, dtype=np.float16)

    in_maps = []
    for c in range(N_CORES):
        sl = np.ascontiguousarray(V[:, c * TSH:(c + 1) * TSH, :])
        in_maps.append({"v": sl, "qwa": qwa_rep, "qwm": qwm_rep,
                       "ident": ident})

    import os
    kw = {}
    if os.environ.get("KERNEL_TRACE"):
        kw = {"trace": True, "tmpdir": os.environ.get("KERNEL_TRACE_DIR")}
    res = run_bass_kernel_spmd(nc, in_maps, list(range(N_CORES)), **kw)
    _CACHE["last_result"] = res

    ha = np.concatenate([res.results[c]["oa"] for c in range(N_CORES)],
                        axis=0).reshape(B, T, D)
    hm = np.concatenate([res.results[c]["om"] for c in range(N_CORES)],
                        axis=0).reshape(B, T, D)
    return (ha, hm)
